# revision 1
# baseline (speedup 1.0000x reference)
"""NT-Xent (SimCLR) contrastive loss on 8 Trainium2 NeuronCores.

Data-parallel: each core owns a 1024-row block of the 2N=8192 rows of z.
Host normalizes z (f32) and hands every core the full z^T in bf16,
rotated so the core's rows sit at columns 0..1023.

Per core, the 1024x8192 logits block is computed as 32 PSUM tiles
[128,2048] (8 row-chunks x 4 col-groups).  PSUM can only be drained by
the ACT and DVE engines, so the exp+rowsum work is split:
  - A-tiles (20): ACT exp(2x) with fused row-sum (accum_out)
  - D-tiles (12, paired per row-chunk): DVE Schraudolph bit-trick exp
    (i16 = rint(G*TA+TB), bitcast bf16), the two tiles of a pair are
    summed elementwise on the otherwise-idle GpSimd, and DVE does one
    bf16 row-sum per pair, staggered off the PSUM critical path.
Positive-pair dots: GpSimd elementwise product + DVE reduce.
lse = ln(rowsum - e^2) on ACT (diagonal removed analytically).
Host: loss = (sum lse - 2 * sum pos_dot) / 8192.
"""

import numpy as np
import ml_dtypes

import concourse.bass as bass
import concourse.bacc as bacc
import concourse.mybir as mybir
import concourse.tile as tile
from concourse.bass_utils import run_bass_kernel_spmd

F32 = mybir.dt.float32
BF16 = mybir.dt.bfloat16
I16 = mybir.dt.int16
I32 = mybir.dt.int32
AF = mybir.ActivationFunctionType
ALU = mybir.AluOpType
AX = mybir.AxisListType

TWO_N = 8192
D = 128
NCORES = 8
E2 = float(np.exp(2.0))

# Schraudolph exp in bf16: i16 = rint(G * TA + TB) bitcast to bf16
# approximates exp(2G); sigma zeroes the mean multiplicative bias.
SIGMA = 0.05753
TA = 256.0 / float(np.log(2.0))
TB = (127.0 - SIGMA) * 128.0
LN2 = float(np.log(2.0))
LC1 = LN2 / float(1 << 23)            # ln-trick scale
LC2 = -(127.0 - SIGMA) * LN2 - 0.0327  # ln-trick bias, calibrated

# AAD repeating pattern: every 3rd tile drained by DVE (10 D, 22 A),
# paced so the PE (the 1.2 GHz column-rate bottleneck) never stalls.
ASSIGN = ["D" if (k % 3 == 2 and k < 29) or k == 30 else "A"
          for k in range(32)]

_CACHE: dict = {}


def _build_program():
    nc = bacc.Bacc(None, target_bir_lowering=False, debug=False)
    zt_d = nc.declare_dram_parameter("zt", [128, TWO_N], BF16, isOutput=False)
    out_d = nc.declare_dram_parameter("out", [128, 2], F32, isOutput=True)

    with tile.TileContext(nc) as tc:
        with (
            tc.tile_pool(name="zt", bufs=1) as zt_pool,
            tc.tile_pool(name="small", bufs=1) as small_pool,
            tc.tile_pool(name="trick", bufs=3) as trick_pool,
            tc.tile_pool(name="psum", bufs=2, space="PSUM") as psum_pool,
        ):
            zt0a = zt_pool.tile([128, 1024], BF16, tag="zt0a", name="zt0a")
            zt0b = zt_pool.tile([128, 1024], BF16, tag="zt0b", name="zt0b")
            zts = [None] + [zt_pool.tile([128, 2048], BF16, tag=f"zt{g}", name=f"zt{g}")
                            for g in range(1, 4)]
            se_act = small_pool.tile([128, 32], F32, tag="se_act")
            se_dve = small_pool.tile([128, 32], F32, tag="se_dve")
            rowS = small_pool.tile([128, 8], F32, tag="rowS")
            lse8 = small_pool.tile([128, 8], F32, tag="lse8")
            sa = small_pool.tile([128, 8], F32, tag="sa")
            sb = small_pool.tile([128, 8], F32, tag="sb")
            outt = small_pool.tile([128, 2], F32, tag="outt")

            # input DMAs: each 2048-col chunk split across both HWDGE rings,
            # zt0 first (it holds lhsT and the first rhs col-group)
            nc.sync.dma_start(zt0a[:], zt_d[:, 0:1024])
            nc.scalar.dma_start(zt0b[:], zt_d[:, 1024:2048])
            for g in range(1, 4):
                nc.sync.dma_start(zts[g][:, 0:1024], zt_d[:, g * 2048:g * 2048 + 1024])
                nc.scalar.dma_start(zts[g][:, 1024:2048],
                                    zt_d[:, g * 2048 + 1024:(g + 1) * 2048])

            nc.gpsimd.memset(se_act[:], 0.0)
            nc.gpsimd.memset(se_dve[:], 0.0)

            for k in range(32):
                r, g = divmod(k, 4)
                lhsT = zt0a[:, r * 128:(r + 1) * 128]
                ps = psum_pool.tile([128, 2048], F32, tag="ps")
                for j in range(4):
                    if g == 0:
                        src_t = zt0a if j < 2 else zt0b
                        rhs = src_t[:, (j % 2) * 512:(j % 2 + 1) * 512]
                    else:
                        rhs = zts[g][:, j * 512:(j + 1) * 512]
                    nc.tensor.matmul(ps[:, j * 512:(j + 1) * 512], lhsT, rhs,
                                     start=True, stop=True)
                if ASSIGN[k] == "A":
                    nc.scalar.activation(ps[:], ps[:], AF.Exp, scale=2.0,
                                         accum_out=se_act[:, k:k + 1])
                else:
                    tr = trick_pool.tile([128, 2048], I16, tag="tr")
                    nc.vector.tensor_scalar(out=tr[:], in0=ps[:],
                                            scalar1=TA, scalar2=TB,
                                            op0=ALU.mult, op1=ALU.add)
                    nc.vector.tensor_reduce(out=se_dve[:, k:k + 1],
                                            in_=tr[:].bitcast(BF16),
                                            axis=AX.X, op=ALU.add)

            # ---- epilogue ----
            nc.vector.tensor_reduce(
                out=sa[:], in_=se_act[:].rearrange("p (r g) -> p r g", g=4),
                axis=AX.X, op=ALU.add)
            nc.vector.tensor_reduce(
                out=sb[:], in_=se_dve[:].rearrange("p (r g) -> p r g", g=4),
                axis=AX.X, op=ALU.add)
            # rowS = (sa - e^2) + sb : diagonal removed analytically
            nc.vector.scalar_tensor_tensor(out=rowS[:], in0=sa[:],
                                           scalar=-E2, in1=sb[:],
                                           op0=ALU.add, op1=ALU.add)
            # ln(x) ~ float(bitcast_i32(x)) * LN2/2^23 - (127-sigma)*LN2
            lnt = small_pool.tile([128, 8], F32, tag="lnt")
            nc.vector.tensor_scalar(out=lnt[:], in0=rowS[:].bitcast(I32),
                                    scalar1=0, scalar2=None,
                                    op0=ALU.add, op1=ALU.bypass)
            nc.vector.tensor_scalar(out=lse8[:], in0=lnt[:],
                                    scalar1=LC1, scalar2=LC2,
                                    op0=ALU.mult, op1=ALU.add)
            nc.vector.tensor_reduce(out=outt[:, 0:1], in_=lse8[:],
                                    axis=AX.X, op=ALU.add)
            nc.vector.memset(outt[:, 1:2], 0.0)
            nc.sync.dma_start(out_d[:], outt[:])

    nc.compile()
    return nc


def _get_program():
    if "nc" not in _CACHE:
        _CACHE["nc"] = _build_program()
    return _CACHE["nc"]


def _prepare_in_maps(emb_i, emb_j):
    z = np.concatenate([np.asarray(emb_i, dtype=np.float32),
                        np.asarray(emb_j, dtype=np.float32)], axis=0)
    zn = z / np.linalg.norm(z, axis=1, keepdims=True)
    znT = np.ascontiguousarray(zn.T)                       # [128, 8192] f32
    in_maps = []
    for c in range(NCORES):
        ztc = np.roll(znT, -1024 * c, axis=1).astype(ml_dtypes.bfloat16)
        in_maps.append({"zt": ztc})
    # positive-pair dot term (O(N*D) input prep, like the normalize)
    pos_sum = float((zn * np.roll(zn, 4096, axis=0)).sum(dtype=np.float64))
    return in_maps, pos_sum


def _execute(in_maps, **kw):
    return run_bass_kernel_spmd(_get_program(), in_maps, list(range(NCORES)), **kw)


def _combine(results, pos_sum):
    lse = 0.0
    for c in range(NCORES):
        o = results[c]["out"].astype(np.float64)
        lse += o[:, 0].sum()
    # pos_logits = dot / TEMPERATURE = 2*dot ; loss = mean(lse - pos)
    return np.array((lse - 2.0 * pos_sum) / TWO_N, dtype=np.float32)


def kernel(emb_i, emb_j):
    in_maps, pos_sum = _prepare_in_maps(emb_i, emb_j)
    res = _execute(in_maps)
    return _combine(res.results, pos_sum)



# revision 2
# speedup vs baseline: 1.0151x; 1.0151x over previous
"""NT-Xent (SimCLR) contrastive loss on 8 Trainium2 NeuronCores — v2.

Exploits the symmetry of z @ z^T: each unordered 1024x1024 block pair of
the 8192x8192 logits matrix is computed ONCE (36 block-units over 8
cores = 4.5 each), halving the exp/drain work vs the data-parallel
baseline.  The mirror contribution of an off-diagonal block is recovered
as per-COLUMN sums of its exp tile, computed on the otherwise-idle PE by
ones-matmuls (4-way column-tiled via tile_position=(0,32j)) reading the
exp tiles back from SBUF.  Row sums come for free: ACT exp with fused
accum_out on even row-chunks; a fused DVE tensor_tensor_reduce over the
Schraudolph-exp tile pair on odd row-chunks.

Per core c (units; rows always 128-partition chunks of 1024-row blocks):
  U0..U3: rows c x cols (c+k)%8, k=0..3  (U0 diag: rowsum only)
  U4: A-cores (c<4): rows (c+4).lo x cols c     (zhalf = z rows (c+4).lo)
      B-cores      : rows c.hi    x cols c-4    (zhalf = z rows c.hi)
All sums are exported raw (row-sum partials [128,16] + column-sum
partials [16,1024] per core); the host assembles S_i = sum_j exp(2 z_i.z_j),
subtracts the diagonal e^2 analytically, and takes mean(log S - pos).
"""

import os
import numpy as np
import ml_dtypes

CS_SERIAL = bool(int(os.environ.get("V2_CS_SERIAL", "0")))  # bisect: M=128 colsums
# odd rows whose pair-add+row-sum runs on GPSIMD instead of DVE
# (empty: walrus rejects TensorScalarPtr on the Pool engine)
GP_STT_ROWS = set(
    int(x) for x in os.environ.get("V2_GP_STT", "").split(",") if x != "")

import concourse.bass as bass
import concourse.bacc as bacc
import concourse.mybir as mybir
import concourse.tile as tile
from concourse.bass_utils import run_bass_kernel_spmd

F32 = mybir.dt.float32
BF16 = mybir.dt.bfloat16
I16 = mybir.dt.int16
AF = mybir.ActivationFunctionType
ALU = mybir.AluOpType
AX = mybir.AxisListType

TWO_N = 8192
D = 128
NCORES = 8
E2 = float(np.exp(2.0))

# Schraudolph exp in bf16: i16 = rint(G * TA + TB) bitcast to bf16
# approximates exp(2G); sigma zeroes the mean multiplicative bias.
SIGMA = 0.05753
TA = 256.0 / float(np.log(2.0))
TB = (127.0 - SIGMA) * 128.0

_CACHE: dict = {}


def _build_program():
    nc = bacc.Bacc(None, target_bir_lowering=False, debug=False)
    zt_d = nc.declare_dram_parameter("zt", [128, 5120], BF16, isOutput=False)
    zh_d = nc.declare_dram_parameter("zh", [128, 512], BF16, isOutput=False)
    out_d = nc.declare_dram_parameter("out", [128, 18], F32, isOutput=True)
    cs_d = nc.declare_dram_parameter("cs", [4, 1024], F32, isOutput=True)

    with tile.TileContext(nc) as tc:
        with (
            tc.tile_pool(name="zt", bufs=1) as zt_pool,
            tc.tile_pool(name="e", bufs=1) as e_pool,
            tc.tile_pool(name="small", bufs=1) as small_pool,
            tc.tile_pool(name="scratch", bufs=2) as scratch_pool,
            tc.tile_pool(name="csb", bufs=2) as csb_pool,
            tc.tile_pool(name="psum", bufs=2, space="PSUM") as psum_pool,
        ):
            zt = zt_pool.tile([128, 5120], BF16, tag="zt", name="zt")
            zh = zt_pool.tile([128, 512], BF16, tag="zh", name="zh")
            se_act = small_pool.tile([128, 14], F32, tag="se_act")
            se_dve = small_pool.tile([128, 4], F32, tag="se_dve")
            ones32 = small_pool.tile([128, 128 if CS_SERIAL else 32], BF16,
                                     tag="ones32")

            # ---- input DMAs: half-group pieces alternate the two HWDGE
            # queues so each group completes in half the single-ring time,
            # in consumption order (g0, g1, ..., zh before g4) ----
            for g in range(4):
                c0 = g * 1024
                nc.sync.dma_start(zt[:, c0:c0 + 512], zt_d[:, c0:c0 + 512])
                nc.scalar.dma_start(zt[:, c0 + 512:c0 + 1024],
                                    zt_d[:, c0 + 512:c0 + 1024])
            nc.sync.dma_start(zh[:], zh_d[:])
            nc.scalar.dma_start(zt[:, 4096:4608], zt_d[:, 4096:4608])
            nc.sync.dma_start(zt[:, 4608:5120], zt_d[:, 4608:5120])

            nc.gpsimd.memset(ones32[:], 1.0)
            nc.gpsimd.memset(se_act[:], 0.0)
            nc.gpsimd.memset(se_dve[:], 0.0)
            # warm the ACT exp table set while input DMAs are in flight
            nc.scalar.activation(se_act[:, 13:14], se_act[:, 13:14],
                                 AF.Exp, scale=0.0)

            # exp tiles kept in SBUF for the column-sum phase
            e_tiles = {}   # (r, d) -> tile  duals [128,2048]
            e_u4 = {}      # q -> tile [128,1024]

            # ---- main stream: logits matmuls + drains ----
            # even rows: both duals ACT (exp + fused accum row-sum)
            # odd rows: both duals DVE Schraudolph, then ONE fused
            # scalar_tensor_tensor pair-add+row-sum per row.
            # U4 between rows 3 and 4 so the final PSUM slot rotation
            # (colsum accumulators) is gated on row-7 drains, not U4.
            def mm_dual(r, d):
                lhsT = zt[:, r * 128:(r + 1) * 128]
                ps = psum_pool.tile([128, 2048], F32, tag="ps")
                for j in range(4):
                    col0 = d * 2048 + j * 512
                    nc.tensor.matmul(ps[:, j * 512:(j + 1) * 512], lhsT,
                                     zt[:, col0:col0 + 512],
                                     start=True, stop=True)
                return ps

            def emit_a0(seg):
                # row 0 in 1024/1024/2048 segments: the first drain only
                # needs group 0, starting ACT ~2.5us into the input DMA
                col0, w, acc = [(0, 1024, 0), (1024, 1024, 1),
                                (2048, 2048, 12)][seg]
                lhsT = zt[:, 0:128]
                ps = psum_pool.tile([128, 2048], F32, tag="ps")
                for j in range(w // 512):
                    nc.tensor.matmul(
                        ps[:, j * 512:(j + 1) * 512], lhsT,
                        zt[:, col0 + j * 512:col0 + (j + 1) * 512],
                        start=True, stop=True)
                e = e_pool.tile([128, w], BF16, tag=f"e0s{acc}",
                                name=f"e0s{acc}")
                nc.scalar.activation(e[:], ps[:, 0:w], AF.Exp, scale=2.0,
                                     accum_out=se_act[:, acc:acc + 1])
                e_tiles[(0, 0, col0)] = e

            def emit_A(r, d):
                ps = mm_dual(r, d)
                e = e_pool.tile([128, 2048], BF16, tag=f"e{r}_{d}",
                                name=f"e{r}_{d}")
                nc.scalar.activation(e[:], ps[:], AF.Exp, scale=2.0,
                                     accum_out=se_act[:, r + d:r + d + 1])
                e_tiles[(r, d)] = e

            def emit_D(r, d):
                ps = mm_dual(r, d)
                e = e_pool.tile([128, 2048], I16, tag=f"e{r}_{d}",
                                name=f"e{r}_{d}")
                nc.vector.tensor_scalar(out=e[:], in0=ps[:],
                                        scalar1=TA, scalar2=TB,
                                        op0=ALU.mult, op1=ALU.add)
                e_tiles[(r, d)] = e
                if d == 1:
                    sc = scratch_pool.tile([128, 2048], BF16, tag="ttr_sc")
                    nc.vector.scalar_tensor_tensor(
                        out=sc[:],
                        in0=e_tiles[(r, 0)][:].bitcast(BF16),
                        scalar=0.0,
                        in1=e_tiles[(r, 1)][:].bitcast(BF16),
                        op0=ALU.add, op1=ALU.add,
                        accum_out=se_dve[:, (r - 1) // 2:(r - 1) // 2 + 1])

            def emit_u4(q):
                lhsT = zh[:, q * 128:(q + 1) * 128]
                ps = psum_pool.tile([128, 2048], F32, tag="ps")
                for j in range(2):
                    nc.tensor.matmul(ps[:, j * 512:(j + 1) * 512], lhsT,
                                     zt[:, 4096 + j * 512:4096 + (j + 1) * 512],
                                     start=True, stop=True)
                e = e_pool.tile([128, 1024], BF16, tag=f"eu4_{q}",
                                name=f"eu4_{q}")
                nc.scalar.activation(e[:], ps[:, 0:1024], AF.Exp, scale=2.0,
                                     accum_out=se_act[:, 8 + q:8 + q + 1])
                e_u4[q] = e

            # Row-pair cadence: even rows drain on ACT (exp + fused
            # accum), odd rows on DVE (Schraudolph + one fused
            # pair-add+row-sum).  With the 2-slot PSUM rotation this
            # overlaps ACT work on row r with DVE work on row r+1; U4
            # (ACT) trails at the end.  Measured best arrangement.
            steps = [("A", 0, 0), ("A", 0, 1), ("D", 1, 0), ("D", 1, 1),
                     ("A", 2, 0), ("A", 2, 1), ("D", 3, 0), ("D", 3, 1),
                     ("A", 4, 0), ("A", 4, 1), ("D", 5, 0), ("D", 5, 1),
                     ("A", 6, 0), ("A", 6, 1), ("D", 7, 0), ("D", 7, 1),
                     ("u", 0), ("u", 1), ("u", 2), ("u", 3)]
            for step in steps:
                if step[0] == "a":
                    emit_a0(step[1])
                elif step[0] == "A":
                    emit_A(step[1], step[2])
                elif step[0] == "D":
                    emit_D(step[1], step[2])
                else:
                    emit_u4(step[1])

            # ---- column sums on PE (4-way column-tiled ones-matmuls) ----
            def e_src(u, ch):
                if u == 4:
                    return e_u4[ch][:, 0:1024]
                d, half = divmod(u, 2)  # u1->(d0,h1) u2->(d1,h0) u3->(d1,h1)
                t = e_tiles[(ch, d)]
                ap = t[:, half * 1024:(half + 1) * 1024]
                if ch % 2 == 1:
                    ap = ap.bitcast(BF16)
                return ap

            # unit u -> column-group strip 32(u-1), col-half (u-1)%2 of a
            # [128,2048] tile (A: units 1,2 / B: units 3,4).  Each unit's
            # accumulation group owns 2 distinct PSUM banks; all four
            # streams run concurrently on the PE via tile_position.
            cs_A = psum_pool.tile([128, 2048], F32, tag="ps")
            cs_B = psum_pool.tile([128, 2048], F32, tag="ps")
            cs_sb = csb_pool.tile([128, 2048], F32, tag="cs_sb")
            upos = {}
            for u in (1, 2, 3, 4):
                j = u - 1
                tile_u = cs_A if u <= 2 else cs_B
                colb = 0 if u % 2 == 1 else 1024
                upos[u] = (tile_u, j, colb)
            nchunks = {1: 8, 2: 8, 3: 8, 4: 4}
            for ci in range(8):
                for u in (1, 2, 3, 4):
                    if ci >= nchunks[u]:
                        continue
                    tile_u, j, colb = upos[u]
                    src = e_src(u, ci)
                    for half in range(2):
                        if CS_SERIAL:
                            out_ap = tile_u[0:128, colb + half * 512:
                                            colb + (half + 1) * 512]
                            kw = {}
                        else:
                            out_ap = tile_u[32 * j:32 * j + 32,
                                            colb + half * 512:
                                            colb + (half + 1) * 512]
                            kw = {"tile_position": (0, 32 * j)}
                        nc.tensor.matmul(
                            out_ap, ones32[:],
                            src[:, half * 512:(half + 1) * 512],
                            start=(ci == 0), stop=(ci == nchunks[u] - 1),
                            **kw)
            for u in (1, 2, 3, 4):
                tile_u, j, colb = upos[u]
                if CS_SERIAL:
                    j = 0
                strip_ps = tile_u[32 * j:32 * j + 32, colb:colb + 1024]
                strip_sb = cs_sb[32 * j:32 * j + 32, colb:colb + 1024]
                if u % 2 == 1:
                    nc.scalar.copy(strip_sb, strip_ps)
                else:
                    nc.vector.tensor_scalar(out=strip_sb, in0=strip_ps,
                                            scalar1=0.0, scalar2=None,
                                            op0=ALU.add, op1=ALU.bypass)
                nc.sync.dma_start(cs_d[u - 1:u, :],
                                  cs_sb[32 * j:32 * j + 1, colb:colb + 1024])

            nc.sync.dma_start(out_d[:, 0:14], se_act[:])
            nc.sync.dma_start(out_d[:, 14:18], se_dve[:])

    nc.compile()
    return nc


def _get_program():
    if "nc" not in _CACHE:
        _CACHE["nc"] = _build_program()
    return _CACHE["nc"]


def _prepare_in_maps(emb_i, emb_j):
    z = np.concatenate([np.asarray(emb_i, dtype=np.float32),
                        np.asarray(emb_j, dtype=np.float32)], axis=0)
    zn = z / np.linalg.norm(z, axis=1, keepdims=True)
    znT = np.ascontiguousarray(zn.T)                       # [128, 8192] f32
    in_maps = []
    for c in range(NCORES):
        groups = [(c + k) % 8 for k in range(4)]
        groups.append(c if c < 4 else (c + 4) % 8)
        ztc = np.concatenate(
            [znT[:, g * 1024:(g + 1) * 1024] for g in groups], axis=1)
        if c < 4:
            zhc = znT[:, (c + 4) * 1024:(c + 4) * 1024 + 512]
        else:
            zhc = znT[:, c * 1024 + 512:(c + 1) * 1024]
        in_maps.append({"zt": ztc.astype(ml_dtypes.bfloat16),
                        "zh": np.ascontiguousarray(zhc).astype(ml_dtypes.bfloat16)})
    # positive-pair dot term (O(N*D) input prep, like the normalize)
    pos_sum = float((zn * np.roll(zn, 4096, axis=0)).sum(dtype=np.float64))
    return in_maps, pos_sum


def _execute(in_maps, **kw):
    return run_bass_kernel_spmd(_get_program(), in_maps, list(range(NCORES)), **kw)


def _combine(results, pos_sum):
    S = np.zeros(TWO_N, dtype=np.float64)
    for c in range(NCORES):
        o = results[c]["out"].astype(np.float64)          # [128, 18]
        cs = results[c]["cs"].astype(np.float64)          # [4, 1024]
        se_act, se_dve = o[:, 0:14], o[:, 14:18]
        rowS = np.empty((8, 128))
        for r in range(8):
            if r == 0:
                rowS[r] = se_act[:, 0] + se_act[:, 1] + se_act[:, 12]
            elif r % 2 == 0:
                rowS[r] = se_act[:, r] + se_act[:, r + 1]
            else:
                rowS[r] = se_dve[:, (r - 1) // 2]
        S[c * 1024:(c + 1) * 1024] += rowS.reshape(-1)    # idx r*128+p
        base = (c + 4) * 1024 if c < 4 else c * 1024 + 512
        S[base:base + 512] += se_act[:, 8:12].T.reshape(-1)  # idx q*128+p
        for u in (1, 2, 3):
            g = (c + u) % 8
            S[g * 1024:(g + 1) * 1024] += cs[u - 1, :]
        g4 = c if c < 4 else c - 4
        S[g4 * 1024:(g4 + 1) * 1024] += cs[3, :]
    S -= E2                                               # remove diagonal
    lse = np.log(S)
    return np.array((lse.sum() - 2.0 * pos_sum) / TWO_N, dtype=np.float32)


def kernel(emb_i, emb_j):
    in_maps, pos_sum = _prepare_in_maps(emb_i, emb_j)
    res = _execute(in_maps)
    return _combine(res.results, pos_sum)
